# revision 4
# baseline (speedup 1.0000x reference)
"""Trainium2 Bass kernel for nn_Attention_15676630631260 (sparse_attention).

reference:
  q = x @ Wq.T + bq ; k = x @ Wk.T + bk ; v = x @ Wv.T + bv        (per batch)
  scores = sigmoid(q @ k.T / sqrt(P))                               [B,S,S]
  out[b,i,j,:] = tril(i,j) * scores[b,i,j] * v[b,j,:]               [B,S,S,P]

B=2, S=512, D=256, P=128.  rel-err gate 2e-2; fp16 end-to-end keeps
rel err ~4.8e-4 and halves every byte vs the 73.4us fp32 predecessor
(measured 45.5us; 8.9 MB/core causal-trimmed fp16 output).

Sharding (8 cores, one NEFF, SPMD): core c -> batch b=c//4, quarter
k=c%4; 8-row groups g=4t+s hold global rows 128t+32s+8k..+8, so the
diag tile of slot s needs only np=32(s+1) partitions on every core.
Odd classes stored j-reversed (flat per-partition DMA bytes).

Compute (fp16 into f32 PSUM, all transpose-free):
  kt[p,j] / qt[p,i] via 2 d-half matmuls + a K=1 bias pass (b x ones);
  scoresT[j,i] = matmul(lhsT=kt, rhs=qt) directly in [j,i] layout;
  ACT sigmoid -> st fp16; only the 32 diag columns are masked (dm) —
  full tiles above the diagonal are provably all-ones.  v[j,p] =
  matmul(lhsT=xT, rhs=WvT) + bias pass -> vrep[j,(p i8)] ACT
  broadcast-copy.

Production: one DVE tensor_tensor per pair-of-groups per j-tile,
out[j,(g2 p i8)] = vrep (unit stride) * st-slice bcast with i INNER
step-1 -> fp16 2x_1P engages (684ns/[128,1024]; the p-inner broadcast
is 1x).  Diagonals: slot-0/1 diag pairs of classes 0+1 and 2+3 are
PARTITION-FUSED into single TTs (bottom 64 partitions = even class,
top 64 = odd class; composite vc/dmc operands built per partition
range) — every element useful, halves diag TT count and removes two
ACT mul streams vs the unfused version.  (14,15) stays on ACT (16
per-row scalar.mul, (i p) sub-layout, scalar-ring DMAs); (2,3),(6,7),
(10,11) are normal DVE pair TTs.  Engine busy: DVE 22.8us, ACT 19.7,
sync ring 23.7 (the binding resource, saturated ~18..40us).

Measured-slower variants, do not revisit blindly: quad-sized full TTs,
scheduler wait-pins forcing emission order, SWDGE (gpsimd) ring for
trailing DMAs (+8us drain), PE-warmup matmuls (128-col matmul cost is
fixed overhead, not pstate), input DMAs split across both HWDGE rings
and/or 512KB full DMAs moved to the scalar ring (46.5 vs 45.5 — ring
moves just shift issue cost onto ACT; the shared SDMA/HBM path is the
cap).  exec = last-DMA-end + ~3.5us receipt + ~2us sem-cleanup on top
of ~7.5us fixed preamble."""

import os
import sys

import numpy as np

for _p in ("/root/.axon_site/_ro/trn_rl_repo", "/opt/trn_rl_repo"):
    if _p not in sys.path and os.path.isdir(_p):
        sys.path.append(_p)

import concourse.bass as bass
import concourse.mybir as mybir
from concourse.tile import TileContext
from concourse import bass_utils

F16 = mybir.dt.float16
F32 = mybir.dt.float32
B, S, D, P = 2, 512, 256, 128
NCORES = 8
GROUP = 8
INV_SQRT_P = float(1.0 / np.sqrt(np.float32(P)))

# crit_a1 [128, 512]: wk0 wk1 xt0h0 xt0h1
# bias3   [1, 384]:   bq bk bv
# crit_a2 [128, 512]: wq0 wq1 xq0 xq1
# crit_a3 [128, 288]: wv0 wv1 mask0
# crit_b  [128, 864]: xt1h0 xt1h1 mask1 xt23h0 xt23h1 mask2 mask3
A1_WK, A1_XT0 = 0, 256
A2_WQ, A2_XQ = 0, 256
A3_WV, A3_MK0 = 0, 256
B_XT1, B_MK1, B_XT23, B_MK2, B_MK3 = 0, 256, 288, 800, 832
# groups whose DIAGONAL tile is produced on ACT ((i p) sub-layout,
# scalar-ring DMA); pair-aligned.  DVE handles everything else.
ACT_DIAG = frozenset({14, 15})
# pair stream order: classes 0,1,3,2
PAIR_ORDER = (0, 2, 4, 6, 12, 14, 8, 10)


def _rows_sel(k: int) -> np.ndarray:
    return np.concatenate(
        [
            np.arange(128 * t + 32 * s + 8 * k, 128 * t + 32 * s + 8 * k + 8)
            for t in range(4)
            for s in range(4)
        ]
    )


def _build_nc() -> bass.Bass:
    nc = bass.Bass(trn_type="TRN2")

    crit_a1 = nc.dram_tensor("crit_a1", [128, 512], F16, kind="ExternalInput")
    bias3 = nc.dram_tensor("bias3", [1, 384], F16, kind="ExternalInput")
    crit_a2 = nc.dram_tensor("crit_a2", [128, 512], F16, kind="ExternalInput")
    crit_a3 = nc.dram_tensor("crit_a3", [128, 288], F16, kind="ExternalInput")
    crit_b = nc.dram_tensor("crit_b", [128, 864], F16, kind="ExternalInput")
    # local output [j_global, g, 1024]; per (tile,group) the 1024 block
    # is (p i8) for DVE-produced tiles, (i8 p) for ACT diag tiles.
    out = nc.dram_tensor("out", [S, 16, GROUP * P], F16, kind="ExternalOutput")

    with TileContext(nc) as tc:
        with (
            tc.tile_pool(name="const", bufs=1) as cpool,
            tc.tile_pool(name="psA", bufs=1, space="PSUM") as psA,
            tc.tile_pool(name="psB", bufs=2, space="PSUM") as psB,
            tc.tile_pool(name="psV", bufs=2, space="PSUM") as psV,
            tc.tile_pool(name="slab", bufs=3) as spool,
        ):
            a1 = cpool.tile([128, 512], F16, tag="a1")
            nc.sync.dma_start(a1[:], crit_a1[:])
            bi = cpool.tile([1, 384], F16, tag="bi")
            nc.sync.dma_start(bi[:], bias3[:])
            a2 = cpool.tile([128, 512], F16, tag="a2")
            nc.sync.dma_start(a2[:], crit_a2[:])
            a3 = cpool.tile([128, 288], F16, tag="a3")
            nc.sync.dma_start(a3[:], crit_a3[:])
            bb = cpool.tile([128, 864], F16, tag="bb")
            nc.sync.dma_start(bb[:], crit_b[:])

            ones = cpool.tile([1, 128], F16, tag="ones")
            nc.vector.memset(ones[:], 1.0)

            bq = bi[0:1, 0:128]
            bk = bi[0:1, 128:256]
            bv = bi[0:1, 256:384]

            def xt_h(c, h):  # x^T [d-half h, j-tile c] fp16 [128,128]
                if c == 0:
                    return a1[:, A1_XT0 + h * 128 : A1_XT0 + (h + 1) * 128]
                if c == 1:
                    return bb[:, B_XT1 + h * 128 : B_XT1 + (h + 1) * 128]
                off = B_XT23 + h * 256 + (c - 2) * 128
                return bb[:, off : off + 128]

            def mk(c):
                off = (A3_MK0, B_MK1, B_MK2, B_MK3)[c]
                t = a3 if c == 0 else bb
                return t[:, off : off + 32]

            ktsb = cpool.tile([128, 512], F16, tag="ktsb")
            qtsb = cpool.tile([128, 128], F16, tag="qtsb")
            st_t = [None] * 4
            dm_t = [None] * 4
            dm32_t = [None] * 4
            vrep_t = [None] * 4
            vt_t = [None] * 4

            vp_t = [None] * 4

            def make_kt(c, cast_on_act=False):
                ktp = psB.tile([128, 128], F32, tag="proj", name=f"ktp{c}")
                nc.tensor.matmul(ktp[:], a1[:, A1_WK : A1_WK + 128], xt_h(c, 0),
                                 start=True, stop=False)
                nc.tensor.matmul(ktp[:], a1[:, A1_WK + 128 : A1_WK + 256], xt_h(c, 1),
                                 start=False, stop=False)
                nc.tensor.matmul(ktp[:], bk, ones[:], start=False, stop=True)
                dst = ktsb[:, c * 128 : (c + 1) * 128]
                if cast_on_act:
                    nc.scalar.copy(dst, ktp[:])
                else:
                    nc.vector.tensor_copy(dst, ktp[:])

            def make_scores(c):
                stp = psB.tile([128, 128], F32, tag="proj", name=f"stp{c}")
                nc.tensor.matmul(stp[:], ktsb[:, c * 128 : (c + 1) * 128], qtsb[:],
                                 start=True, stop=True)
                st = cpool.tile([128, 128], F16, tag=f"st{c}")
                nc.scalar.activation(
                    st[:], stp[:], mybir.ActivationFunctionType.Sigmoid,
                    scale=INV_SQRT_P,
                )
                st_t[c] = st
                if c != 3:
                    dm = cpool.tile([128, 32], F16, tag=f"dm{c}")
                    nc.vector.tensor_mul(dm[:], st[:, 32 * c : 32 * c + 32], mk(c))
                    dm_t[c] = dm
                if any((4 * c + ss) in ACT_DIAG for ss in range(4)):
                    dm32 = cpool.tile([128, 32], F32, tag=f"dm32_{c}")
                    nc.vector.tensor_mul(dm32[:], st[:, 32 * c : 32 * c + 32], mk(c))
                    dm32_t[c] = dm32

            def make_v(c):
                vp = psV.tile([128, 128], F32, tag="vp", name=f"vp{c}")
                nc.tensor.matmul(vp[:], xt_h(c, 0), a3[:, A3_WV : A3_WV + 128],
                                 start=True, stop=False)
                nc.tensor.matmul(vp[:], xt_h(c, 1), a3[:, A3_WV + 128 : A3_WV + 256],
                                 start=False, stop=False)
                nc.tensor.matmul(vp[:], ones[:], bv, start=False, stop=True)
                vp_t[c] = vp
                if c != 3:
                    vrep = cpool.tile([128, 1024], F16, tag=f"vrep{c}")
                    nc.scalar.copy(
                        vrep[:].rearrange("j (p i) -> j p i", i=GROUP),
                        vp[:].unsqueeze(2).broadcast_to([128, 128, GROUP]),
                    )
                    vrep_t[c] = vrep
                if any((4 * c + ss) in ACT_DIAG for ss in range(4)):
                    vt = cpool.tile([128, 128], F16, tag=f"vt{c}")
                    nc.scalar.copy(vt[:], vp[:])
                    vt_t[c] = vt

            out_r = out.rearrange("(t j) g x -> j t (g x)", j=128)

            def emit_pair(glo, skip_diag=False):
                L = glo // 4 + 1
                c = L - 1
                s_lo = glo % 4
                slab = spool.tile(
                    [128, L * 2048], F16, tag=f"slab{L}", name=f"slab_p{glo}"
                )
                for jt in range(L - 1):
                    sec = slab[:, jt * 2048 : (jt + 1) * 2048]
                    nc.vector.tensor_mul(
                        sec.rearrange("q (g p i) -> q g p i", g=2, i=GROUP),
                        vrep_t[jt][:]
                        .rearrange("j (p i) -> j p i", i=GROUP)
                        .unsqueeze(1)
                        .broadcast_to([128, 2, 128, GROUP]),
                        st_t[jt][:, glo * 8 : (glo + 2) * 8]
                        .rearrange("j (g i) -> j g i", g=2)
                        .unsqueeze(2)
                        .broadcast_to([128, 2, 128, GROUP]),
                    )
                    nc.sync.dma_start(
                        out_r[:, jt : jt + 1, glo * 1024 : (glo + 2) * 1024],
                        sec.rearrange("q (t x) -> q t x", t=1),
                    )
                if skip_diag:
                    return
                # diagonal pair tile
                sec = slab[:, (L - 1) * 2048 : L * 2048]
                act = glo in ACT_DIAG
                # compute on all 128 partitions (the mask zeroes rows
                # beyond each group's allowance; engine time is
                # free-axis-bound anyway) — only the DMAs trim.
                if act:
                    for gg in range(2):
                        for ii in range(GROUP):
                            col = (glo + gg) * 8 + ii - 32 * c
                            off = gg * 1024 + ii * 128
                            nc.scalar.mul(
                                sec[:, off : off + 128],
                                vt_t[c][:],
                                mul=dm32_t[c][:, col : col + 1],
                            )
                else:
                    nc.vector.tensor_mul(
                        sec.rearrange("q (g p i) -> q g p i", g=2, i=GROUP),
                        vrep_t[c][:]
                        .rearrange("j (p i) -> j p i", i=GROUP)
                        .unsqueeze(1)
                        .broadcast_to([128, 2, 128, GROUP]),
                        dm_t[c][:, 8 * s_lo : 8 * s_lo + 16]
                        .rearrange("j (g i) -> j g i", g=2)
                        .unsqueeze(2)
                        .broadcast_to([128, 2, 128, GROUP]),
                    )
                ring = nc.scalar if act else nc.sync
                for gg in range(2):
                    np_g = 32 * (s_lo + gg + 1)
                    psg = slice(128 - np_g, 128) if c % 2 else slice(0, np_g)
                    soff = (L - 1) * 2048 + gg * 1024
                    ring.dma_start(
                        out_r[psg, c : c + 1, (glo + gg) * 1024 : (glo + gg + 1) * 1024],
                        slab[psg, soff : soff + 1024].rearrange(
                            "q (t x) -> q t x", t=1
                        ),
                    )


            def build_vc(name, vp_lo, vp_hi):
                # composite v-broadcast: bottom 64 partitions from the
                # even class, top 64 from the odd class (slot-0/1 diag
                # allowances never cross the halfway line)
                vc = cpool.tile([128, 1024], F16, tag=name)
                nc.scalar.copy(
                    vc[0:64, :].rearrange("j (p i) -> j p i", i=GROUP),
                    vp_lo[0:64, :].unsqueeze(2).broadcast_to([64, 128, GROUP]),
                )
                nc.scalar.copy(
                    vc[64:128, :].rearrange("j (p i) -> j p i", i=GROUP),
                    vp_hi[64:128, :].unsqueeze(2).broadcast_to([64, 128, GROUP]),
                )
                return vc

            def build_dmc(name, c_lo, c_hi):
                dmc = cpool.tile([128, 16], F16, tag=name)
                lo0 = 32 * c_lo
                hi0 = 32 * c_hi
                nc.vector.tensor_mul(
                    dmc[0:64, :], st_t[c_lo][0:64, lo0 : lo0 + 16],
                    mk(c_lo)[0:64, 0:16],
                )
                nc.vector.tensor_mul(
                    dmc[64:128, :], st_t[c_hi][64:128, hi0 : hi0 + 16],
                    mk(c_hi)[64:128, 0:16],
                )
                return dmc

            def emit_fused_diag(qa, qb, vc, dmc):
                # one TT computes the slot-0/1 diag pairs of BOTH fused
                # classes: bottom 64 partitions hold groups (qa,qa+1),
                # top 64 hold (qb,qb+1) — every element is useful.
                ca, cb = qa // 4, qb // 4
                slab = spool.tile([128, 2048], F16, tag="fslab", name=f"fslab{qa}")
                nc.vector.tensor_mul(
                    slab[:].rearrange("q (g p i) -> q g p i", g=2, i=GROUP),
                    vc[:]
                    .rearrange("j (p i) -> j p i", i=GROUP)
                    .unsqueeze(1)
                    .broadcast_to([128, 2, 128, GROUP]),
                    dmc[:]
                    .rearrange("j (g i) -> j g i", g=2)
                    .unsqueeze(2)
                    .broadcast_to([128, 2, 128, GROUP]),
                )
                for gg in range(2):
                    np_g = 32 * (gg + 1)
                    sec = slab[:, gg * 1024 : (gg + 1) * 1024]
                    nc.sync.dma_start(
                        out_r[0:np_g, ca : ca + 1,
                              (qa + gg) * 1024 : (qa + gg + 1) * 1024],
                        sec[0:np_g, :].rearrange("q (t x) -> q t x", t=1),
                    )
                    nc.sync.dma_start(
                        out_r[128 - np_g : 128, cb : cb + 1,
                              (qb + gg) * 1024 : (qb + gg + 1) * 1024],
                        sec[128 - np_g : 128, :].rearrange("q (t x) -> q t x", t=1),
                    )

            # ---- ramp ----
            make_kt(0)
            with tc.tile_wait_until(0.002):
                qtp = psA.tile([128, 128], F32, tag="qtp")
                nc.tensor.matmul(qtp[:], a2[:, A2_WQ : A2_WQ + 128],
                                 a2[:, A2_XQ : A2_XQ + 128], start=True, stop=False)
                nc.tensor.matmul(qtp[:], a2[:, A2_WQ + 128 : A2_WQ + 256],
                                 a2[:, A2_XQ + 128 : A2_XQ + 256], start=False,
                                 stop=False)
                nc.tensor.matmul(qtp[:], bq, ones[:], start=False, stop=True)
                nc.vector.tensor_copy(qtsb[:], qtp[:])
            with tc.tile_wait_until(0.0025):
                make_scores(0)
                make_v(0)
            emit_pair(2)
            with tc.tile_wait_until(0.0035):
                make_kt(1, cast_on_act=True)
                make_scores(1)
                make_v(1)
                vc01 = build_vc("vc01", vp_t[0], vp_t[1])
                dmc01 = build_dmc("dmc01", 0, 1)
            emit_pair(4, skip_diag=True)
            emit_fused_diag(0, 4, vc01, dmc01)
            emit_pair(6)
            for c in (2, 3):
                make_kt(c, cast_on_act=True)
                make_scores(c)
                make_v(c)
            vc23 = build_vc("vc23", vp_t[2], vp_t[3])
            dmc23 = build_dmc("dmc23", 2, 3)
            emit_pair(12, skip_diag=True)
            emit_pair(14)
            emit_fused_diag(8, 12, vc23, dmc23)
            emit_pair(8, skip_diag=True)
            emit_pair(10)

    _split_multi_waits(nc)
    return nc


def _split_multi_waits(nc):
    """This toolchain's walrus accepts at most one sync wait per
    instruction; split extras into single-wait NoOps just before the
    instruction on the same engine queue (waits are ANDed preconditions,
    executed in order on the engine's queue — semantically identical)."""
    for fn in nc.m.functions:
        for blk in fn.blocks:
            insts = blk.instructions
            i = 0
            while i < len(insts):
                inst = insts[i]
                si = getattr(inst, "sync_info", None)
                if si is not None and si.on_wait is not None and len(si.on_wait) > 1:
                    waits = list(si.on_wait)
                    nops = [
                        mybir.InstNoOp(
                            name=nc.get_next_instruction_name(),
                            engine=inst.engine,
                            sync_info=mybir.SyncInfo(on_wait=[w], on_update=[]),
                            bass_nofuse=True,
                        )
                        for w in waits[:-1]
                    ]
                    si.on_wait = [waits[-1]]
                    insts[i:i] = nops
                    i += len(nops)
                i += 1


_NC_CACHE = None


def _get_nc():
    global _NC_CACHE
    if _NC_CACHE is None:
        _NC_CACHE = _build_nc()
    return _NC_CACHE


def _in_maps(x_set, Wq, bq, Wk, bk, Wv, bv):
    f16 = np.float16
    xts = [np.ascontiguousarray(x_set[b].T).astype(f16) for b in range(B)]
    wqT = np.ascontiguousarray(Wq.T).astype(f16)
    wkT = np.ascontiguousarray(Wk.T).astype(f16)
    wvT = np.ascontiguousarray(Wv.T).astype(f16)
    b3 = np.concatenate([bq, bk, bv]).astype(f16)[None, :]
    jj = np.arange(128)
    maps = []
    for cidx in range(NCORES):
        b, k = divmod(cidx, 4)
        rows = _rows_sel(k)
        xtT = xts[b]
        xqT = xtT[:, rows]
        # per-class 32-col masks (cols = this class's local rows); odd
        # classes j-reversed
        mask = np.empty((4, 128, 32), f16)
        for c in range(4):
            m = ((c * 128 + jj)[:, None] <= rows[None, 32 * c : 32 * c + 32])
            if c % 2:
                m = m[::-1]
            mask[c] = m.astype(f16)
        xw = np.concatenate([xtT[:, 256:384], xtT[:, 511:383:-1]], axis=1)
        crit_a1 = np.concatenate(
            [wkT[0:128], wkT[128:256], xtT[0:128, 0:128], xtT[128:256, 0:128]],
            axis=1,
        )
        crit_a2 = np.concatenate(
            [wqT[0:128], wqT[128:256], xqT[0:128], xqT[128:256]], axis=1
        )
        crit_a3 = np.concatenate([wvT[0:128], wvT[128:256], mask[0]], axis=1)
        crit_b = np.concatenate(
            [
                xtT[0:128, 255:127:-1], xtT[128:256, 255:127:-1],
                mask[1],
                xw[0:128], xw[128:256],
                mask[2], mask[3],
            ],
            axis=1,
        )
        maps.append(
            {
                "crit_a1": np.ascontiguousarray(crit_a1),
                "bias3": np.ascontiguousarray(b3),
                "crit_a2": np.ascontiguousarray(crit_a2),
                "crit_a3": np.ascontiguousarray(crit_a3),
                "crit_b": np.ascontiguousarray(crit_b),
            }
        )
    return maps


def run(x_set, Wq, bq, Wk, bk, Wv, bv, **spmd_kwargs):
    nc = _get_nc()
    in_maps = _in_maps(x_set, Wq, bq, Wk, bk, Wv, bv)
    res = bass_utils.run_bass_kernel_spmd(
        nc, in_maps, core_ids=list(range(NCORES)), **spmd_kwargs
    )
    full = np.zeros((B, S, S, P), np.float32)
    fc = np.zeros((128, S, P), np.float32)
    for cidx in range(NCORES):
        b, k = divmod(cidx, 4)
        o = res.results[cidx]["out"].astype(np.float32).reshape(4, 128, 16, 1024)
        o[1] = o[1][::-1]
        o[3] = o[3][::-1]
        fc[:] = 0.0
        for g in range(16):
            L = g // 4 + 1
            for t in range(L):
                blk = o[t, :, g, :]
                if t == L - 1 and g in ACT_DIAG:
                    b3v = blk.reshape(128, GROUP, 128)  # [j, i, p]
                else:
                    b3v = blk.reshape(128, 128, GROUP).transpose(0, 2, 1)
                fc[g * 8 : (g + 1) * 8, t * 128 : (t + 1) * 128, :] = b3v.transpose(
                    1, 0, 2
                )
        full[b, _rows_sel(k)] = fc
    return full, res


def kernel(x_set, Wq, bq, Wk, bk, Wv, bv):
    full, _ = run(x_set, Wq, bq, Wk, bk, Wv, bv)
    return full


# revision 5
# speedup vs baseline: 1.0296x; 1.0296x over previous
"""Trainium2 Bass kernel for nn_Attention_15676630631260 (sparse_attention).

reference:
  q = x @ Wq.T + bq ; k = x @ Wk.T + bk ; v = x @ Wv.T + bv        (per batch)
  scores = sigmoid(q @ k.T / sqrt(P))                               [B,S,S]
  out[b,i,j,:] = tril(i,j) * scores[b,i,j] * v[b,j,:]               [B,S,S,P]

B=2, S=512, D=256, P=128.  rel-err gate 2e-2; fp16 end-to-end keeps
rel err ~4.8e-4 and halves every byte vs the 73.4us fp32 predecessor
(measured 45.5us; 8.9 MB/core causal-trimmed fp16 output).

Sharding (8 cores, one NEFF, SPMD): core c -> batch b=c//4, quarter
k=c%4; 8-row groups g=4t+s hold global rows 128t+32s+8k..+8, so the
diag tile of slot s needs only np=32(s+1) partitions on every core.
Odd classes stored j-reversed (flat per-partition DMA bytes).

Compute (fp16 into f32 PSUM, all transpose-free):
  kt[p,j] / qt[p,i] via 2 d-half matmuls + a K=1 bias pass (b x ones);
  scoresT[j,i] = matmul(lhsT=kt, rhs=qt) directly in [j,i] layout;
  ACT sigmoid -> st fp16; only the 32 diag columns are masked (dm) —
  full tiles above the diagonal are provably all-ones.  v[j,p] =
  matmul(lhsT=xT, rhs=WvT) + bias pass -> vrep[j,(p i8)] ACT
  broadcast-copy.

Production: one DVE tensor_tensor per pair-of-groups per j-tile,
out[j,(g2 p i8)] = vrep (unit stride) * st-slice bcast with i INNER
step-1 -> fp16 2x_1P engages (684ns/[128,1024]; the p-inner broadcast
is 1x).  Diagonals: slot-0/1 diag pairs of classes 0+1 and 2+3 are
PARTITION-FUSED into single TTs (bottom 64 partitions = even class,
top 64 = odd class; composite vc/dmc operands built per partition
range) — every element useful, halves diag TT count and removes two
ACT mul streams vs the unfused version.  (14,15) stays on ACT (16
per-row scalar.mul, (i p) sub-layout, scalar-ring DMAs); (2,3),(6,7),
(10,11) are normal DVE pair TTs.  Engine busy: DVE 22.8us, ACT 19.7,
sync ring 23.7 (the binding resource, saturated ~18..40us).

Measured-slower variants, do not revisit blindly: quad-sized full TTs,
scheduler wait-pins forcing emission order, SWDGE (gpsimd) ring for
trailing DMAs (+8us drain), PE-warmup matmuls (128-col matmul cost is
fixed overhead, not pstate), input DMAs split across both HWDGE rings
and/or 512KB full DMAs moved to the scalar ring (46.5 vs 45.5 — ring
moves just shift issue cost onto ACT; the shared SDMA/HBM path is the
cap).  Also measured-slower at equal clocks: a2/bias inputs on the
scalar ring + lowered pins + endgame (10,11)-diag/class-2-full DMAs on
scalar (47.8 vs 45.5 — the scalar FIFO stalls behind the ACT-mul-gated
(14,15) DMAs, and resequencing the ramp shuffles the whole schedule).
exec = last-DMA-end + ~3.5us receipt + ~2us sem-cleanup on top of
~7.5us fixed preamble.  NOTE: the device intermittently runs ~20%
clock-throttled (pair-TT duration 1450ns vs 1216ns at full speed) —
always check a TT duration as a clock probe before comparing exec
times across runs; full-clock samples for THIS kernel: 45489, 46630,
46751, 47314."""

import os
import sys

import numpy as np

for _p in ("/root/.axon_site/_ro/trn_rl_repo", "/opt/trn_rl_repo"):
    if _p not in sys.path and os.path.isdir(_p):
        sys.path.append(_p)

import concourse.bass as bass
import concourse.mybir as mybir
from concourse.tile import TileContext
from concourse import bass_utils

F16 = mybir.dt.float16
F32 = mybir.dt.float32
B, S, D, P = 2, 512, 256, 128
NCORES = 8
GROUP = 8
INV_SQRT_P = float(1.0 / np.sqrt(np.float32(P)))

# crit_a1 [128, 512]: wk0 wk1 xt0h0 xt0h1
# bias3   [1, 384]:   bq bk bv
# crit_a2 [128, 512]: wq0 wq1 xq0 xq1
# crit_a3 [128, 288]: wv0 wv1 mask0
# crit_b  [128, 864]: xt1h0 xt1h1 mask1 xt23h0 xt23h1 mask2 mask3
A1_WK, A1_XT0 = 0, 256
A2_WQ, A2_XQ = 0, 256
A3_WV, A3_MK0 = 0, 256
B_XT1, B_MK1, B_XT23, B_MK2, B_MK3 = 0, 256, 288, 800, 832
# groups whose DIAGONAL tile is produced on ACT ((i p) sub-layout,
# scalar-ring DMA); pair-aligned.  DVE handles everything else.
ACT_DIAG = frozenset({14, 15})
# pair stream order: classes 0,1,3,2
PAIR_ORDER = (0, 2, 4, 6, 12, 14, 8, 10)


def _rows_sel(k: int) -> np.ndarray:
    return np.concatenate(
        [
            np.arange(128 * t + 32 * s + 8 * k, 128 * t + 32 * s + 8 * k + 8)
            for t in range(4)
            for s in range(4)
        ]
    )


def _build_nc() -> bass.Bass:
    nc = bass.Bass(trn_type="TRN2")

    crit_a1 = nc.dram_tensor("crit_a1", [128, 512], F16, kind="ExternalInput")
    bias3 = nc.dram_tensor("bias3", [1, 384], F16, kind="ExternalInput")
    crit_a2 = nc.dram_tensor("crit_a2", [128, 512], F16, kind="ExternalInput")
    crit_a3 = nc.dram_tensor("crit_a3", [128, 288], F16, kind="ExternalInput")
    crit_b = nc.dram_tensor("crit_b", [128, 864], F16, kind="ExternalInput")
    # local output [j_global, g, 1024]; per (tile,group) the 1024 block
    # is (p i8) for DVE-produced tiles, (i8 p) for ACT diag tiles.
    out = nc.dram_tensor("out", [S, 16, GROUP * P], F16, kind="ExternalOutput")

    with TileContext(nc) as tc:
        with (
            tc.tile_pool(name="const", bufs=1) as cpool,
            tc.tile_pool(name="psA", bufs=1, space="PSUM") as psA,
            tc.tile_pool(name="psB", bufs=2, space="PSUM") as psB,
            tc.tile_pool(name="psV", bufs=2, space="PSUM") as psV,
            tc.tile_pool(name="slab", bufs=3) as spool,
        ):
            a1 = cpool.tile([128, 512], F16, tag="a1")
            nc.sync.dma_start(a1[:], crit_a1[:])
            bi = cpool.tile([1, 384], F16, tag="bi")
            nc.sync.dma_start(bi[:], bias3[:])
            a2 = cpool.tile([128, 512], F16, tag="a2")
            nc.sync.dma_start(a2[:], crit_a2[:])
            a3 = cpool.tile([128, 288], F16, tag="a3")
            nc.sync.dma_start(a3[:], crit_a3[:])
            bb = cpool.tile([128, 864], F16, tag="bb")
            nc.sync.dma_start(bb[:], crit_b[:])

            ones = cpool.tile([1, 128], F16, tag="ones")
            nc.vector.memset(ones[:], 1.0)

            bq = bi[0:1, 0:128]
            bk = bi[0:1, 128:256]
            bv = bi[0:1, 256:384]

            def xt_h(c, h):  # x^T [d-half h, j-tile c] fp16 [128,128]
                if c == 0:
                    return a1[:, A1_XT0 + h * 128 : A1_XT0 + (h + 1) * 128]
                if c == 1:
                    return bb[:, B_XT1 + h * 128 : B_XT1 + (h + 1) * 128]
                off = B_XT23 + h * 256 + (c - 2) * 128
                return bb[:, off : off + 128]

            def mk(c):
                off = (A3_MK0, B_MK1, B_MK2, B_MK3)[c]
                t = a3 if c == 0 else bb
                return t[:, off : off + 32]

            ktsb = cpool.tile([128, 512], F16, tag="ktsb")
            qtsb = cpool.tile([128, 128], F16, tag="qtsb")
            st_t = [None] * 4
            dm_t = [None] * 4
            dm32_t = [None] * 4
            vrep_t = [None] * 4
            vt_t = [None] * 4

            vp_t = [None] * 4

            def make_kt(c, cast_on_act=False):
                ktp = psB.tile([128, 128], F32, tag="proj", name=f"ktp{c}")
                nc.tensor.matmul(ktp[:], a1[:, A1_WK : A1_WK + 128], xt_h(c, 0),
                                 start=True, stop=False)
                nc.tensor.matmul(ktp[:], a1[:, A1_WK + 128 : A1_WK + 256], xt_h(c, 1),
                                 start=False, stop=False)
                nc.tensor.matmul(ktp[:], bk, ones[:], start=False, stop=True)
                dst = ktsb[:, c * 128 : (c + 1) * 128]
                if cast_on_act:
                    nc.scalar.copy(dst, ktp[:])
                else:
                    nc.vector.tensor_copy(dst, ktp[:])

            def make_scores(c):
                stp = psB.tile([128, 128], F32, tag="proj", name=f"stp{c}")
                nc.tensor.matmul(stp[:], ktsb[:, c * 128 : (c + 1) * 128], qtsb[:],
                                 start=True, stop=True)
                st = cpool.tile([128, 128], F16, tag=f"st{c}")
                nc.scalar.activation(
                    st[:], stp[:], mybir.ActivationFunctionType.Sigmoid,
                    scale=INV_SQRT_P,
                )
                st_t[c] = st
                if c != 3:
                    dm = cpool.tile([128, 32], F16, tag=f"dm{c}")
                    nc.vector.tensor_mul(dm[:], st[:, 32 * c : 32 * c + 32], mk(c))
                    dm_t[c] = dm
                if any((4 * c + ss) in ACT_DIAG for ss in range(4)):
                    dm32 = cpool.tile([128, 32], F32, tag=f"dm32_{c}")
                    nc.vector.tensor_mul(dm32[:], st[:, 32 * c : 32 * c + 32], mk(c))
                    dm32_t[c] = dm32

            def make_v(c):
                vp = psV.tile([128, 128], F32, tag="vp", name=f"vp{c}")
                nc.tensor.matmul(vp[:], xt_h(c, 0), a3[:, A3_WV : A3_WV + 128],
                                 start=True, stop=False)
                nc.tensor.matmul(vp[:], xt_h(c, 1), a3[:, A3_WV + 128 : A3_WV + 256],
                                 start=False, stop=False)
                nc.tensor.matmul(vp[:], ones[:], bv, start=False, stop=True)
                vp_t[c] = vp
                if c != 3:
                    vrep = cpool.tile([128, 1024], F16, tag=f"vrep{c}")
                    nc.scalar.copy(
                        vrep[:].rearrange("j (p i) -> j p i", i=GROUP),
                        vp[:].unsqueeze(2).broadcast_to([128, 128, GROUP]),
                    )
                    vrep_t[c] = vrep
                if any((4 * c + ss) in ACT_DIAG for ss in range(4)):
                    vt = cpool.tile([128, 128], F16, tag=f"vt{c}")
                    nc.scalar.copy(vt[:], vp[:])
                    vt_t[c] = vt

            out_r = out.rearrange("(t j) g x -> j t (g x)", j=128)

            def emit_pair(glo, skip_diag=False):
                L = glo // 4 + 1
                c = L - 1
                s_lo = glo % 4
                slab = spool.tile(
                    [128, L * 2048], F16, tag=f"slab{L}", name=f"slab_p{glo}"
                )
                for jt in range(L - 1):
                    sec = slab[:, jt * 2048 : (jt + 1) * 2048]
                    nc.vector.tensor_mul(
                        sec.rearrange("q (g p i) -> q g p i", g=2, i=GROUP),
                        vrep_t[jt][:]
                        .rearrange("j (p i) -> j p i", i=GROUP)
                        .unsqueeze(1)
                        .broadcast_to([128, 2, 128, GROUP]),
                        st_t[jt][:, glo * 8 : (glo + 2) * 8]
                        .rearrange("j (g i) -> j g i", g=2)
                        .unsqueeze(2)
                        .broadcast_to([128, 2, 128, GROUP]),
                    )
                    nc.sync.dma_start(
                        out_r[:, jt : jt + 1, glo * 1024 : (glo + 2) * 1024],
                        sec.rearrange("q (t x) -> q t x", t=1),
                    )
                if skip_diag:
                    return
                # diagonal pair tile
                sec = slab[:, (L - 1) * 2048 : L * 2048]
                act = glo in ACT_DIAG
                # compute on all 128 partitions (the mask zeroes rows
                # beyond each group's allowance; engine time is
                # free-axis-bound anyway) — only the DMAs trim.
                if act:
                    for gg in range(2):
                        for ii in range(GROUP):
                            col = (glo + gg) * 8 + ii - 32 * c
                            off = gg * 1024 + ii * 128
                            nc.scalar.mul(
                                sec[:, off : off + 128],
                                vt_t[c][:],
                                mul=dm32_t[c][:, col : col + 1],
                            )
                else:
                    nc.vector.tensor_mul(
                        sec.rearrange("q (g p i) -> q g p i", g=2, i=GROUP),
                        vrep_t[c][:]
                        .rearrange("j (p i) -> j p i", i=GROUP)
                        .unsqueeze(1)
                        .broadcast_to([128, 2, 128, GROUP]),
                        dm_t[c][:, 8 * s_lo : 8 * s_lo + 16]
                        .rearrange("j (g i) -> j g i", g=2)
                        .unsqueeze(2)
                        .broadcast_to([128, 2, 128, GROUP]),
                    )
                ring = nc.scalar if act else nc.sync
                for gg in range(2):
                    np_g = 32 * (s_lo + gg + 1)
                    psg = slice(128 - np_g, 128) if c % 2 else slice(0, np_g)
                    soff = (L - 1) * 2048 + gg * 1024
                    ring.dma_start(
                        out_r[psg, c : c + 1, (glo + gg) * 1024 : (glo + gg + 1) * 1024],
                        slab[psg, soff : soff + 1024].rearrange(
                            "q (t x) -> q t x", t=1
                        ),
                    )


            def build_vc(name, vp_lo, vp_hi):
                # composite v-broadcast: bottom 64 partitions from the
                # even class, top 64 from the odd class (slot-0/1 diag
                # allowances never cross the halfway line)
                vc = cpool.tile([128, 1024], F16, tag=name)
                nc.scalar.copy(
                    vc[0:64, :].rearrange("j (p i) -> j p i", i=GROUP),
                    vp_lo[0:64, :].unsqueeze(2).broadcast_to([64, 128, GROUP]),
                )
                nc.scalar.copy(
                    vc[64:128, :].rearrange("j (p i) -> j p i", i=GROUP),
                    vp_hi[64:128, :].unsqueeze(2).broadcast_to([64, 128, GROUP]),
                )
                return vc

            def build_dmc(name, c_lo, c_hi):
                dmc = cpool.tile([128, 16], F16, tag=name)
                lo0 = 32 * c_lo
                hi0 = 32 * c_hi
                nc.vector.tensor_mul(
                    dmc[0:64, :], st_t[c_lo][0:64, lo0 : lo0 + 16],
                    mk(c_lo)[0:64, 0:16],
                )
                nc.vector.tensor_mul(
                    dmc[64:128, :], st_t[c_hi][64:128, hi0 : hi0 + 16],
                    mk(c_hi)[64:128, 0:16],
                )
                return dmc

            def emit_fused_diag(qa, qb, vc, dmc):
                # one TT computes the slot-0/1 diag pairs of BOTH fused
                # classes: bottom 64 partitions hold groups (qa,qa+1),
                # top 64 hold (qb,qb+1) — every element is useful.
                ca, cb = qa // 4, qb // 4
                slab = spool.tile([128, 2048], F16, tag="fslab", name=f"fslab{qa}")
                nc.vector.tensor_mul(
                    slab[:].rearrange("q (g p i) -> q g p i", g=2, i=GROUP),
                    vc[:]
                    .rearrange("j (p i) -> j p i", i=GROUP)
                    .unsqueeze(1)
                    .broadcast_to([128, 2, 128, GROUP]),
                    dmc[:]
                    .rearrange("j (g i) -> j g i", g=2)
                    .unsqueeze(2)
                    .broadcast_to([128, 2, 128, GROUP]),
                )
                for gg in range(2):
                    np_g = 32 * (gg + 1)
                    sec = slab[:, gg * 1024 : (gg + 1) * 1024]
                    nc.sync.dma_start(
                        out_r[0:np_g, ca : ca + 1,
                              (qa + gg) * 1024 : (qa + gg + 1) * 1024],
                        sec[0:np_g, :].rearrange("q (t x) -> q t x", t=1),
                    )
                    nc.sync.dma_start(
                        out_r[128 - np_g : 128, cb : cb + 1,
                              (qb + gg) * 1024 : (qb + gg + 1) * 1024],
                        sec[128 - np_g : 128, :].rearrange("q (t x) -> q t x", t=1),
                    )

            # ---- ramp ----
            make_kt(0)
            with tc.tile_wait_until(0.002):
                qtp = psA.tile([128, 128], F32, tag="qtp")
                nc.tensor.matmul(qtp[:], a2[:, A2_WQ : A2_WQ + 128],
                                 a2[:, A2_XQ : A2_XQ + 128], start=True, stop=False)
                nc.tensor.matmul(qtp[:], a2[:, A2_WQ + 128 : A2_WQ + 256],
                                 a2[:, A2_XQ + 128 : A2_XQ + 256], start=False,
                                 stop=False)
                nc.tensor.matmul(qtp[:], bq, ones[:], start=False, stop=True)
                nc.vector.tensor_copy(qtsb[:], qtp[:])
            with tc.tile_wait_until(0.0025):
                make_scores(0)
                make_v(0)
            emit_pair(2)
            with tc.tile_wait_until(0.0035):
                make_kt(1, cast_on_act=True)
                make_scores(1)
                make_v(1)
                vc01 = build_vc("vc01", vp_t[0], vp_t[1])
                dmc01 = build_dmc("dmc01", 0, 1)
            emit_pair(4, skip_diag=True)
            emit_fused_diag(0, 4, vc01, dmc01)
            emit_pair(6)
            for c in (2, 3):
                make_kt(c, cast_on_act=True)
                make_scores(c)
                make_v(c)
            vc23 = build_vc("vc23", vp_t[2], vp_t[3])
            dmc23 = build_dmc("dmc23", 2, 3)
            emit_pair(12, skip_diag=True)
            emit_pair(14)
            emit_fused_diag(8, 12, vc23, dmc23)
            emit_pair(8, skip_diag=True)
            emit_pair(10)

    _split_multi_waits(nc)
    return nc


def _split_multi_waits(nc):
    """This toolchain's walrus accepts at most one sync wait per
    instruction; split extras into single-wait NoOps just before the
    instruction on the same engine queue (waits are ANDed preconditions,
    executed in order on the engine's queue — semantically identical)."""
    for fn in nc.m.functions:
        for blk in fn.blocks:
            insts = blk.instructions
            i = 0
            while i < len(insts):
                inst = insts[i]
                si = getattr(inst, "sync_info", None)
                if si is not None and si.on_wait is not None and len(si.on_wait) > 1:
                    waits = list(si.on_wait)
                    nops = [
                        mybir.InstNoOp(
                            name=nc.get_next_instruction_name(),
                            engine=inst.engine,
                            sync_info=mybir.SyncInfo(on_wait=[w], on_update=[]),
                            bass_nofuse=True,
                        )
                        for w in waits[:-1]
                    ]
                    si.on_wait = [waits[-1]]
                    insts[i:i] = nops
                    i += len(nops)
                i += 1


_NC_CACHE = None


def _get_nc():
    global _NC_CACHE
    if _NC_CACHE is None:
        _NC_CACHE = _build_nc()
    return _NC_CACHE


def _in_maps(x_set, Wq, bq, Wk, bk, Wv, bv):
    f16 = np.float16
    xts = [np.ascontiguousarray(x_set[b].T).astype(f16) for b in range(B)]
    wqT = np.ascontiguousarray(Wq.T).astype(f16)
    wkT = np.ascontiguousarray(Wk.T).astype(f16)
    wvT = np.ascontiguousarray(Wv.T).astype(f16)
    b3 = np.concatenate([bq, bk, bv]).astype(f16)[None, :]
    jj = np.arange(128)
    maps = []
    for cidx in range(NCORES):
        b, k = divmod(cidx, 4)
        rows = _rows_sel(k)
        xtT = xts[b]
        xqT = xtT[:, rows]
        # per-class 32-col masks (cols = this class's local rows); odd
        # classes j-reversed
        mask = np.empty((4, 128, 32), f16)
        for c in range(4):
            m = ((c * 128 + jj)[:, None] <= rows[None, 32 * c : 32 * c + 32])
            if c % 2:
                m = m[::-1]
            mask[c] = m.astype(f16)
        xw = np.concatenate([xtT[:, 256:384], xtT[:, 511:383:-1]], axis=1)
        crit_a1 = np.concatenate(
            [wkT[0:128], wkT[128:256], xtT[0:128, 0:128], xtT[128:256, 0:128]],
            axis=1,
        )
        crit_a2 = np.concatenate(
            [wqT[0:128], wqT[128:256], xqT[0:128], xqT[128:256]], axis=1
        )
        crit_a3 = np.concatenate([wvT[0:128], wvT[128:256], mask[0]], axis=1)
        crit_b = np.concatenate(
            [
                xtT[0:128, 255:127:-1], xtT[128:256, 255:127:-1],
                mask[1],
                xw[0:128], xw[128:256],
                mask[2], mask[3],
            ],
            axis=1,
        )
        maps.append(
            {
                "crit_a1": np.ascontiguousarray(crit_a1),
                "bias3": np.ascontiguousarray(b3),
                "crit_a2": np.ascontiguousarray(crit_a2),
                "crit_a3": np.ascontiguousarray(crit_a3),
                "crit_b": np.ascontiguousarray(crit_b),
            }
        )
    return maps


def run(x_set, Wq, bq, Wk, bk, Wv, bv, **spmd_kwargs):
    nc = _get_nc()
    in_maps = _in_maps(x_set, Wq, bq, Wk, bk, Wv, bv)
    res = bass_utils.run_bass_kernel_spmd(
        nc, in_maps, core_ids=list(range(NCORES)), **spmd_kwargs
    )
    full = np.zeros((B, S, S, P), np.float32)
    fc = np.zeros((128, S, P), np.float32)
    for cidx in range(NCORES):
        b, k = divmod(cidx, 4)
        o = res.results[cidx]["out"].astype(np.float32).reshape(4, 128, 16, 1024)
        o[1] = o[1][::-1]
        o[3] = o[3][::-1]
        fc[:] = 0.0
        for g in range(16):
            L = g // 4 + 1
            for t in range(L):
                blk = o[t, :, g, :]
                if t == L - 1 and g in ACT_DIAG:
                    b3v = blk.reshape(128, GROUP, 128)  # [j, i, p]
                else:
                    b3v = blk.reshape(128, 128, GROUP).transpose(0, 2, 1)
                fc[g * 8 : (g + 1) * 8, t * 128 : (t + 1) * 128, :] = b3v.transpose(
                    1, 0, 2
                )
        full[b, _rows_sel(k)] = fc
    return full, res


def kernel(x_set, Wq, bq, Wk, bk, Wv, bv):
    full, _ = run(x_set, Wq, bq, Wk, bk, Wv, bv)
    return full
